# revision 37
# baseline (speedup 1.0000x reference)
"""Trainium2 Bass kernel: 16-head MHA (B=2, T=2048, D=1024, d_k=64).

Sharding (8 NeuronCores): data-parallel over the batch (2) x tensor-parallel
over head groups (4 groups of 4 heads).  Core c handles batch b = c//4 and
heads [4g, 4g+4) with g = c%4.  Each core computes its partial output
    sum_{h in group} softmax((q Wq_h + bq_h)(k Wk_h)^T / 8) (v Wv_h) Wo_h
and the host sums the 4 partials per batch and adds the constant row
bo + bv @ Wo once.  bk is dropped: with the all-ones mask it shifts every
score row by a per-row constant, which softmax ignores exactly.

Design notes (420us baseline -> ~211us):
  * every matmul operand is bf16 (FWL weight loads, fp32 PSUM
    accumulate); output DMA'd as bf16 and upconverted host-side.
    The two heads of a pair sit on partition halves 0:64 / 64:128, so
    their C=64 scores matmuls land on disjoint PE row-tiles (T0/T8) and
    execute CONCURRENTLY (measured: starts 3 ns apart) -- scores cost
    half the naive streaming time.  (Column-tiled pair splits of the
    C=128 projections were tried and serialize; only row tiles overlap.)
  * V is projected directly in [t, v-col] layout (stationary = x^T
    chunk, moving = Wv) -- no PE transposes.  Activations arrive via
    host-packed chunk-major layouts (xq/xk per 512-col group, xv per
    128-row k-tile) so each projection unit depends on ~1 MB of DMA,
    not the whole tensor.
  * attention runs in 512-wide q stripes; per (stripe, head-pair, kt):
    2 concurrent scores MMs -> one [128,1024] fp32 PSUM tile, one ACT
    exp -> bf16 es, 2 aV MMs accumulating into per-head [65,512] PSUM
    (the 65th V_ext ones-column yields softmax rowsums for free, which
    is column-optimal: a separate rowsum matmul would stream es again).
    PSUM: scores dbuf 2x2 banks + O-accum 2 + filler 2 = 8 exactly.
  * one flat software-pipelined (stripe, hp, kt) stream: aV runs 1-3
    steps behind scores/exp (deepened at block starts) so the FIFO PE
    queue never waits on an exp or on the previous block's drains.
  * the PE is kept dense -- and the HAM clock gate at 2.4 GHz -- by
    emitting everything else as fillers inside the ACT-bound attention
    loop: K groups 1-3 + V tiles 6-15 + Q stripe s+1 projections, the
    reciprocal dance, and stripe s-1's output projection, each unit
    ordered so its drain is emitted before its first consumer.
    ~150 dependency-free warm-up matmuls cover the ~7us DMA startup.
  * softmax denominators: rowsum rows are collected at partitions 32h;
    one batched ACT Ln + Exp(-1) computes all reciprocals directly on
    that layout (ACT cost depends only on per-lane depth, so no
    transposes are needed), then one C=128 selector matmul per head
    pair broadcasts them across partitions.  The last stripe runs
    hp0's reciprocal under hp1's attention and drains on the
    otherwise-idle ACT queue to shorten the serial tail.

Session 2 (209.9us -> ~204us) landed: DVE reciprocal offload for the
mid-stream stripes (nc.vector.reciprocal f32->bf16 under
allow_low_precision, popped at deque slot 0 with bcasts ~9 slots
later; ACT Ln+Exp kept for the latency-critical last-stripe dances),
m1 q-projections carried one stripe later (stripe 0 is ~6us
PE-overloaded structurally), halved wq/xq0/wk/xk0 transfers with
half-wave phase-A projection (first exp 31.7 -> 29.6us), rowsum rows
drained before u rows in the last block, warm2 sized to the dance.

Structure (trace-measured): first exp ~29.6us | 128 exps at ~1.12us
cadence, stripes 1-3 fully dense, stripe-0 ~13us of ACT gaps (~6us
structural PE overload + ~7us DMA-arrival JIT misses) | tail ~14us.
ACT busy ~148us IS the floor: (N+352)/1.2 ns per ACTIVATE, so 128
[128,1024] exps can't go below ~147us.

DEAD ENDS (all HW-measured, do not retry):
  * DMA multi-queue: sync+scalar HWDGE and gpsimd SWDGE queues all
    share ~220GB/s with PACKET round-robin -- queue choice cannot
    express priority, so any concurrent queue dilutes the critical
    phase-A set.  2-way dealt, 3-way dealt, critical-first-per-queue,
    and gated-SWDGE side channels measured +4..+29us vs one sync
    queue in strict need order.  Completion sems also fire ~5-7us
    after a transfer's last packet under load.
  * fp8 (e4m3) anywhere in the multiply chain: measured 7.0e-2 rel
    err (gate 2e-2).  Softmax/projection weight noise does NOT
    sqrt(N)-average: perturbing weights against random values lands
    the full ~4-5% element noise directly on the output.  This also
    kills fp8-DoubleRow aV/es plans.
  * [128,2048] exps (to amortize the +352): needs 4-bank scores
    tiles double-buffered = 8 banks; O-accum (2, ones-column rowsum
    trick needs M=65 so two heads cannot share a bank) + fillers (2)
    don't fit.  Single-buffered 4-bank scores serializes PE vs ACT
    (period 2.85us/2kt vs 2.24) -- worse.
  * merging the two per-hp OT normalize muls into one [128,512] op
    via a combined ub tile (hh1 drained to partitions 64:128 with a
    partition-SHIFTED tensor_copy): correct but +16us -- the shifted
    copy/mul lowers to something far slower than the aligned form.
    Keep drains partition-aligned; only the dst may be offset (the
    existing OT-mul / rowsum-copy patterns).

OPEN LEAD (big, unverified, ~15us): stage scores out of PSUM with
GPSIMD (idle all run) tensor_copy fp32->fp16 [128,1024] tiles into
an SBUF ring, then exp in [128,4096] batches (32x(4096+352)/1.2 =
119us of ACT vs 147).  fp16 staging keeps logit error ~0.4%; bf16 is
too coarse (3%).  Needs: gpsimd PSUM-read capability + per-op cost
verified, +~30KB/partition SBUF (es tiles grow to batch width),
aV flush lag raised to a full batch, and accepts stripes 1-3
becoming PE-bound at ~31us/stripe (net ~-15us).
"""

import functools
import os
from collections import deque

import ml_dtypes
import numpy as np

import concourse.bass as bass
import concourse.mybir as mybir
import concourse.tile as tile
from concourse import bacc
from concourse.bass_utils import run_bass_kernel_spmd

F32 = mybir.dt.float32
F32R = mybir.dt.float32r
BF16 = mybir.dt.bfloat16
F8 = mybir.dt.float8e4
AFT = mybir.ActivationFunctionType
BF = ml_dtypes.bfloat16
WSCALE = 1.0  # (fp8 input quantization was tried: ~7% output error --
#  softmax/projection weight noise does NOT sqrt-average away; any fp8
#  tensor in the multiply chain lands ~5-7% directly on the output)

D = 1024          # model dim
T = 2048          # sequence length
B = 2             # batch
HEADS = 16        # total heads
DK = 64           # head dim
NCORES = 8
GH = 4            # heads per core
GD = GH * DK      # 256 projection cols per core
NF = D // 128     # 8 contraction chunks
NKT = T // 128    # 16 k tiles
NQS = 4           # 512-wide q stripes
QW = T // NQS     # 512
SCALE = 1.0 / np.sqrt(np.float32(DK))  # 1/8

# Results of the last run (for test harness introspection: exec_time_ns etc.)
LAST_RESULTS = None


@functools.lru_cache(maxsize=1)
def _build_program():
    nc = bacc.Bacc("TRN2", target_bir_lowering=False, debug=False,
                   num_devices=NCORES)

    # host-packed activation layouts (see _pack_* in kernel()):
    #   xq[s]  = [128, NF*QW]  q-stripe s, chunk-major (8 KiB DMA lines)
    #   xk[qh] = [128, NF*QW]  k column-group qh, chunk-major
    #   xv[tb] = [128, NF*128] k-tile tb, chunk-major (2 KiB lines)
    xq = nc.declare_dram_parameter("xq", [NQS, 128, NF * QW], BF16,
                                   isOutput=False)
    xk = nc.declare_dram_parameter("xk", [4, 128, NF * QW], BF16,
                                   isOutput=False)
    xv = nc.declare_dram_parameter("xv", [NKT, 128, NF * 128], BF16,
                                   isOutput=False)
    wq = nc.declare_dram_parameter("wq", [128, NF * GD], BF16, isOutput=False)
    wk = nc.declare_dram_parameter("wk", [128, NF * GD], BF16, isOutput=False)
    wv = nc.declare_dram_parameter("wv", [128, NF * GD], BF16, isOutput=False)
    wo = nc.declare_dram_parameter("wo", [2, 128, D], BF16, isOutput=False)
    bqv = nc.declare_dram_parameter("bqv", [128, 2], F32, isOutput=False)
    out = nc.declare_dram_parameter("out", [T, D], BF16, isOutput=True)

    import contextlib
    with tile.TileContext(nc) as tc, contextlib.ExitStack() as _st:
        # ---- persistent pools -------------------------------------------
        def _pool(**kw):
            return _st.enter_context(tc.tile_pool(**kw))

        if True:
            kt_pool = _pool(name="kt", bufs=2)
            vext_pool = _pool(name="vext", bufs=NKT)
            qts_pool = _pool(name="qts", bufs=NQS * 2)
            ots_pool = _pool(name="ots", bufs=NQS * 2)
            w_pool = _pool(name="wts", bufs=3)
            wo_pool = _pool(name="wop", bufs=2)
            xq_pool = _pool(name="xq", bufs=NQS)
            xk_pool = _pool(name="xk", bufs=4)
            xv_pool = _pool(name="xv", bufs=NKT)
            const_pool = _pool(name="const", bufs=1)
            ones_f32 = const_pool.tile([128, GH], F32, tag="ones32")
            nc.gpsimd.memset(ones_f32[:], 1.0)
            ones_bf = const_pool.tile([128, DK], BF16, tag="onesbf")
            nc.gpsimd.memset(ones_bf[:], 1.0)
            bqv_sb = const_pool.tile([128, 2], F32, tag="bqv")

            KT = [kt_pool.tile([128, T], BF16, tag="kt", name=f"kt{m}")
                  for m in range(2)]
            VE = [vext_pool.tile([128, GH * (DK + 1)], BF16, tag="vext",
                                 name=f"ve{i}") for i in range(NKT)]
            # per-stripe Q^T and O^T tiles (heads of pair hp stacked 64+64)
            QTs = [[qts_pool.tile([128, QW], BF16, tag="qts",
                                  name=f"qt{s}_{m}") for m in range(2)]
                   for s in range(NQS)]
            OTs = [[ots_pool.tile([128, QW], BF16, tag="ots",
                                  name=f"ot{s}_{m}") for m in range(2)]
                   for s in range(NQS)]
            WO = [wo_pool.tile([128, D], BF16, tag="wop", name=f"wo{m}")
                  for m in range(2)]

            wq_sb = w_pool.tile([128, NF * GD], BF16, tag="w", name="wq_sb")
            wk_sb = w_pool.tile([128, NF * GD], BF16, tag="w", name="wk_sb")
            wv_sb = w_pool.tile([128, NF * GD], BF16, tag="w", name="wv_sb")
            XQs = [xq_pool.tile([128, NF * QW], BF16, tag="xq",
                                name=f"xqs{s}") for s in range(NQS)]
            XKq = [xk_pool.tile([128, NF * QW], BF16, tag="xk",
                                name=f"xkq{i}") for i in range(4)]
            XVt = [xv_pool.tile([128, NF * 128], BF16, tag="xv",
                                name=f"xvt{i}") for i in range(NKT)]

            # ---- DMA in, spread over all three DMA-capable queues so the
            # startup-critical 5 MB (wq,xq0,wk,xk0,wv,xv0-5 -- everything
            # phase A gates on) streams at aggregate HBM rate instead of
            # serializing one queue's per-transfer ~2us completion costs.
            #   scalar HWDGE: weights + even xv tiles (idle until the exp
            #     stream starts, which is long after these land)
            #   sync HWDGE:   activations needed by phase A + stripe 0
            #   gpsimd SWDGE: background tensors needed late (bqv by the
            #     Q0 drain, xq1 by stripe-0 hp1 fillers, wo by stripe 1)
            # SDMA engines round-robin the ACTIVE queues at packet
            # granularity, so queue choice cannot express priority --
            # whatever is enqueued anywhere flows concurrently.  Priority
            # therefore comes from ORDER, exactly like the baseline's
            # single queue; the second HWDGE queue (scalar -- idle until
            # the exp stream starts, and all issues happen up front) only
            # serves to overlap per-transfer fixed costs.  Transfers are
            # dealt to the two queues alternately in global need order so
            # arrival order tracks need order at ~2x effective rate.
            # (A 3-way split with bulk data on the gpsimd SWDGE queue was
            # tried and regressed 25us: everything round-robins, so the
            # phase-A set finished only when ALL front-loaded bytes did.)
            # All bulk transfers ride ONE sync-engine HWDGE queue in strict
            # need order.  Multi-queue variants (2-way and 3-way splits,
            # critical-first per queue, need-order dealing) were all tried
            # and regressed 11-29us: concurrent queues share the same
            # ~220 GB/s and round-robin at packet granularity, so extra
            # queues only dilute priority -- a single busy queue already
            # pipelines transfers through the 8 completion-sem lanes.
            # The first four transfers are halved so phase A's Q0/K0
            # projections pipeline under the second halves' transfers.
            HW = NF * QW // 2     # half of a chunk-major activation tile
            HG = NF * GD // 2     # half of a chunk-major weight tile
            nc.sync.dma_start(wq_sb[:, 0:HG], wq[:, 0:HG])
            nc.sync.dma_start(XQs[0][:, 0:HW], xq[0][:, 0:HW])
            nc.sync.dma_start(wk_sb[:, 0:HG], wk[:, 0:HG])
            nc.sync.dma_start(XKq[0][:, 0:HW], xk[0][:, 0:HW])
            nc.sync.dma_start(wq_sb[:, HG:], wq[:, HG:])
            nc.sync.dma_start(XQs[0][:, HW:], xq[0][:, HW:])
            nc.sync.dma_start(wk_sb[:, HG:], wk[:, HG:])
            nc.sync.dma_start(XKq[0][:, HW:], xk[0][:, HW:])
            nc.sync.dma_start(wv_sb[:], wv[:])
            for tb in range(6):
                nc.sync.dma_start(XVt[tb][:], xv[tb])
            nc.sync.dma_start(XKq[1][:], xk[1])
            for tb in range(6, 10):
                nc.sync.dma_start(XVt[tb][:], xv[tb])
            nc.sync.dma_start(XKq[2][:], xk[2])
            nc.sync.dma_start(XKq[3][:], xk[3])
            for tb in range(10, NKT):
                nc.sync.dma_start(XVt[tb][:], xv[tb])
            nc.sync.dma_start(XQs[1][:], xq[1])
            nc.sync.dma_start(WO[0][:], wo[0])
            nc.sync.dma_start(WO[1][:], wo[1])
            nc.sync.dma_start(XQs[2][:], xq[2])
            nc.sync.dma_start(XQs[3][:], xq[3])
            # tiny, and off the bulk queue's serial order.  (A gated
            # SWDGE side-channel for late tensors was tried twice: even
            # gated to start after phase A, its packets round-robin
            # against the sync queue's stripe-0 feed and cost ~4us.)
            nc.gpsimd.dma_start(bqv_sb[:], bqv[:])

            # head-pair selectors: sel[hp][c, m] = (c == 32*(hp*2 + m//64)),
            # i.e. r_bc[m, :] = rinvT[32*head(m), :] after the C=128 matmul
            sel = [const_pool.tile([128, 128], BF16, tag=f"sel{hp}",
                                   name=f"sel{hp}") for hp in range(2)]
            for hp in range(2):
                nc.gpsimd.memset(sel[hp][:], 0.0)
                for hh in range(2):
                    c = 32 * (hp * 2 + hh)
                    nc.vector.tensor_copy(
                        sel[hp][c:c + 1, hh * DK:(hh + 1) * DK],
                        ones_bf[0:1, 0:DK])

            # V_ext ones columns (persistent; written once, no DMA dep)
            for tb in range(NKT):
                ve_r = VE[tb][:].rearrange("p (h x) -> p h x", x=DK + 1)
                nc.vector.tensor_copy(
                    ve_r[:, :, DK:DK + 1],
                    ones_f32[:].rearrange("p (h x) -> p h x", x=1))

            # (A same-bank row-tiled pair variant of these projections --
            # start=False accumulation onto a pre-zeroed bank -- was
            # tried and hangs the device at runtime; keep full-C MMs.)
            def q_project(s, m, ps_q, fc):
                nc.tensor.matmul(
                    ps_q[:],
                    wq_sb[:, fc * GD + m * 128:fc * GD + (m + 1) * 128],
                    XQs[s][:, fc * QW:(fc + 1) * QW],
                    start=(fc == 0), stop=(fc == NF - 1))
                if fc == NF - 1:
                    nc.vector.tensor_scalar_add(
                        QTs[s][m][:], ps_q[:], bqv_sb[:, m:m + 1])

            def k_project(qh, m, ps_k, fc):
                nc.tensor.matmul(
                    ps_k[:],
                    wk_sb[:, fc * GD + m * 128:fc * GD + (m + 1) * 128],
                    XKq[qh][:, fc * QW:(fc + 1) * QW],
                    start=(fc == 0), stop=(fc == NF - 1))
                if fc == NF - 1:
                    nc.vector.tensor_copy(
                        KT[m][:, qh * QW:(qh + 1) * QW], ps_k[:])

            def v_project(tb, ps_v, dc):
                nc.tensor.matmul(
                    ps_v[:, 0:GD],
                    XVt[tb][:, dc * 128:(dc + 1) * 128],
                    wv_sb[:, dc * GD:(dc + 1) * GD],
                    start=(dc == 0), stop=(dc == NF - 1))
                if dc == NF - 1:
                    ve_r = VE[tb][:].rearrange("p (h x) -> p h x", x=DK + 1)
                    nc.vector.tensor_copy(
                        ve_r[:, :, 0:DK],
                        ps_v[:, 0:GD].rearrange("p (h x) -> p h x", x=DK))

            # ---- phase A: warm the HAM clock gate with dummy matmuls
            # (no DMA dependency), then project Q stripe 0, K group 0 and
            # V tiles 0-5 as their inputs land.
            with tc.tile_pool(name="psA", bufs=8,
                              space=bass.MemorySpace.PSUM) as psA:
                # ~6us of dependency-free matmuls: warms the HAM clock
                # gate AND covers the DMA/preamble startup so real
                # projections start the moment their data lands.  Q0/K0
                # run in half-tile waves matching the split DMAs above,
                # so projection overlaps the second half's transfer.
                warm = psA.tile([128, QW], F32, tag="psA", name="warm")
                for i in range(110):
                    nc.tensor.matmul(
                        warm[0:DK, 0:DK], ones_bf[:, 0:DK],
                        ones_bf[:, 0:DK], start=True, stop=True)
                ps_q = [psA.tile([128, QW], F32, tag="psA", name=f"psq{m}")
                        for m in range(2)]
                ps_k = [psA.tile([128, QW], F32, tag="psA", name=f"psk0_{m}")
                        for m in range(2)]
                for half in range(2):
                    for m in range(2):
                        for fc in range(4 * half, 4 * half + 4):
                            q_project(0, m, ps_q[m], fc)
                for half in range(2):
                    for m in range(2):
                        for fc in range(4 * half, 4 * half + 4):
                            k_project(0, m, ps_k[m], fc)
                for tb in range(6):
                    ps_v = psA.tile([128, QW], F32, tag="psA",
                                    name=f"psv{tb}")
                    for dc in range(NF):
                        v_project(tb, ps_v, dc)

            # ---- phase B: striped attention with PE fillers -------------
            with contextlib.ExitStack() as _stB:
                def _poolB(**kw):
                    return _stB.enter_context(tc.tile_pool(**kw))

                es_pool = _poolB(name="ep", bufs=5)
                ub_pool = _poolB(name="ubp", bufs=8)
                rs_pool = _poolB(name="rsp", bufs=2)
                ob_pool = _poolB(name="obp", bufs=4)
                psS = _poolB(name="psS", bufs=2,
                             space=bass.MemorySpace.PSUM)
                psO = _poolB(name="psO", bufs=2,
                             space=bass.MemorySpace.PSUM)
                psF = _poolB(name="psF", bufs=2,
                             space=bass.MemorySpace.PSUM)
                ub_tiles = {}     # (qs, hp, hh) -> [64, 512] f32 tile
                rs_tiles = {}     # qs -> [128, 512] f32 rowsum-spread tile

                fstate = {}

                def qproj_fillers(s):
                    fs = []
                    for m in range(2):
                        def mk(mm, fc):
                            def f():
                                if fc == 0:
                                    fstate['q', mm] = psF.tile(
                                        [128, QW], F32, tag="psF",
                                        name=f"psq{s}_{mm}")
                                q_project(s, mm, fstate['q', mm], fc)
                            return f
                        for fc in range(NF):
                            fs.append(mk(m, fc))
                    return fs

                def kq_fillers(qh, m):
                    """K projection of column-group qh, head-pair tile m
                    (2 chunk-pairs per filler)."""
                    def mk(fp):
                        def f():
                            if fp == 0:
                                fstate['k', qh, m] = psF.tile(
                                    [128, QW], F32, tag="psF",
                                    name=f"psk{qh}_{m}")
                            for fc in (2 * fp, 2 * fp + 1):
                                k_project(qh, m, fstate['k', qh, m], fc)
                        return f
                    return [mk(fp) for fp in range(4)]

                def vtb_fillers(tb):
                    """V projection of k-tile tb (4 chunk-pairs/filler)."""
                    def mk(dp):
                        def f():
                            if dp == 0:
                                fstate['v', tb] = psF.tile(
                                    [128, QW], F32, tag="psF",
                                    name=f"psv{tb}")
                            for dc in range(4 * dp, 4 * dp + 4):
                                v_project(tb, fstate['v', tb], dc)
                        return f
                    return [mk(0), mk(1)]

                def recip_fillers(s, hps=(0, 1), dve=False):
                    """Reciprocal + normalize for stripe s (rowsums at
                    partitions 32h of rs_tiles[s]).  ACT cost depends only
                    on per-lane depth (512), so Ln/Exp run directly on the
                    [32h, q] layout and the selector matmul consumes the
                    result as-is -- no transposes needed.  The caller must
                    space fs[0] a few pops after the rowsum drains.

                    dve=True computes the reciprocal on the DVE instead
                    (iterative divide, ~2-4us/tile) -- the DVE is ~50%
                    idle mid-kernel while the ACT queue is the bottleneck,
                    so stripes consumed mid-stream use this path and keep
                    the ACT Ln+Exp only for the latency-critical
                    last-stripe dances.  The caller must emit the bcast
                    fillers several pops after fs[0] to cover the divide
                    latency."""
                    fs = []

                    if dve:
                        def t2():
                            rinv = rs_pool.tile([128, QW], BF16, tag="rinv",
                                                name=f"rinv{s}")
                            with nc.allow_low_precision(
                                    reason="softmax denom bf16"):
                                nc.vector.reciprocal(rinv[:], rs_tiles[s][:])
                            recip_fillers.rinvT = rinv
                    else:
                        def t2():
                            lnr = rs_pool.tile([128, QW], F32, tag="rsT",
                                               name=f"lnr{s}")
                            nc.scalar.activation(lnr[:], rs_tiles[s][:],
                                                 AFT.Ln)
                            rinv = rs_pool.tile([128, QW], BF16, tag="rinv",
                                                name=f"rinv{s}")
                            nc.scalar.activation(rinv[:], lnr[:],
                                                 AFT.Exp, scale=-1.0)
                            recip_fillers.rinvT = rinv
                    fs.append(t2)

                    def mk_bcast(hp):
                        def f():
                            r_bc = psF.tile([128, QW], F32, tag="psF",
                                            name=f"rbc{s}_{hp}")
                            nc.tensor.matmul(
                                r_bc[:],
                                sel[hp][:],
                                recip_fillers.rinvT[:],
                                start=True, stop=True)
                            for hh in range(2):
                                nc.vector.tensor_mul(
                                    OTs[s][hp][hh * DK:(hh + 1) * DK, :],
                                    ub_tiles.pop((s, hp, hh))[0:DK, :],
                                    r_bc[hh * DK:(hh + 1) * DK, :])
                        return f
                    for hp in hps:
                        fs.append(mk_bcast(hp))
                    return fs
                recip_fillers.rinvT = None

                def outproj_fillers(s):
                    """Each (tt, ei) unit is split into a matmul closure
                    and a drain closure so the PE filler bursts stay
                    fine-grained inside the exp-bound attention cadence."""
                    fs = []

                    def mk_mm(tt, ei):
                        def f():
                            if ei == 0:
                                outproj_fillers.ob = ob_pool.tile(
                                    [128, D], BF16, tag="ob",
                                    name=f"ob{s}_{tt}")
                            f_ps = psF.tile([128, QW], F32, tag="psF",
                                            name=f"fps{s}_{tt}_{ei}")
                            fstate['op'] = f_ps
                            for m in range(2):
                                nc.tensor.matmul(
                                    f_ps[:],
                                    OTs[s][m][:, tt * 128:(tt + 1) * 128],
                                    WO[m][:, ei * QW:(ei + 1) * QW],
                                    start=(m == 0), stop=(m == 1))
                        return f

                    def mk_drain(tt, ei):
                        def f():
                            ob = outproj_fillers.ob
                            f_ps = fstate.pop('op')
                            if s == NQS - 1 and (tt + ei) % 2 == 0:
                                # tail: alternate drains between the idle
                                # ACT queue and DVE so they run 2-wide
                                nc.scalar.activation(
                                    ob[:, ei * QW:(ei + 1) * QW], f_ps[:],
                                    AFT.Copy)
                            else:
                                nc.vector.tensor_copy(
                                    ob[:, ei * QW:(ei + 1) * QW], f_ps[:])
                            t0 = (s * 4 + tt) * 128
                            if s == NQS - 1:
                                # tail: per-half DMAs fire right after
                                # their own drain, so the final transfer
                                # (which gates kernel end) starts earlier
                                nc.sync.dma_start(
                                    out[t0:t0 + 128, ei * QW:(ei + 1) * QW],
                                    ob[:, ei * QW:(ei + 1) * QW])
                            elif ei == 1:
                                nc.sync.dma_start(out[t0:t0 + 128, :], ob[:])
                        return f
                    for tt in range(4):
                        for ei in range(2):
                            fs.append(mk_mm(tt, ei))
                            fs.append(mk_drain(tt, ei))
                    return fs
                outproj_fillers.ob = None

                # flat (qs, hp, kt) stream: aV is emitted 1-3 steps behind
                # scores/exp so the FIFO PE queue never waits on an exp
                # before issuing independent scores work.  At block starts
                # the hold-back deepens to 3 so the previous block's DVE
                # drains (which gate aV(kt0) via o_ps buffer reuse) finish
                # under the run-ahead scores instead of stalling the PE.
                fillers = deque()
                pending = deque()  # (qs, hp, o_ps, es, kt)

                def flush_one():
                    pqs, php, po_ps, pes, pkt = pending.popleft()
                    for hh in range(2):
                        h = php * 2 + hh
                        nc.tensor.matmul(
                            po_ps[hh][0:DK + 1, :],
                            VE[pkt][:, h * (DK + 1):(h + 1) * (DK + 1)],
                            pes[:, hh * QW:(hh + 1) * QW],
                            start=(pkt == 0), stop=(pkt == NKT - 1))
                    if pkt == NKT - 1:
                        # drain O^T + rowsum row; heads at partitions 32h.
                        # The very last block's drains go on the otherwise
                        # idle ACT queue to shorten the serial tail.
                        last = pqs == NQS - 1 and php == 1
                        if last:
                            # rowsum rows first (they gate the tail Ln),
                            # u drains after, 2-wide on DVE + idle ACT.
                            us = [ub_pool.tile([128, QW], F32, tag="ub",
                                               name=f"ub{pqs}_{php}_{hh}")
                                  for hh in range(2)]
                            h0, h1 = php * 2, php * 2 + 1
                            nc.vector.tensor_copy(
                                rs_tiles[pqs][32 * h0:32 * h0 + 1, :],
                                po_ps[0][DK:DK + 1, :])
                            nc.scalar.activation(
                                rs_tiles[pqs][32 * h1:32 * h1 + 1, :],
                                po_ps[1][DK:DK + 1, :], AFT.Copy)
                            nc.vector.tensor_copy(
                                us[0][0:DK, :], po_ps[0][0:DK, :])
                            nc.scalar.activation(
                                us[1][0:DK, :], po_ps[1][0:DK, :],
                                AFT.Copy)
                            for hh in range(2):
                                ub_tiles[(pqs, php, hh)] = us[hh]
                        else:
                            for hh in range(2):
                                h = php * 2 + hh
                                u = ub_pool.tile([128, QW], F32, tag="ub",
                                                 name=f"ub{pqs}_{php}_{hh}")
                                nc.vector.tensor_copy(
                                    u[0:DK, :], po_ps[hh][0:DK, :])
                                nc.vector.tensor_copy(
                                    rs_tiles[pqs][32 * h:32 * h + 1, :],
                                    po_ps[hh][DK:DK + 1, :])
                                ub_tiles[(pqs, php, hh)] = u
                        if pqs == NQS - 1 and php == 0:
                            # last stripe: overlap hp0's half of the
                            # reciprocal under hp1's attention
                            fillers.extend([spacer] * 3)
                            fillers.extend(recip_fillers(pqs, hps=(0,)))

                def spacer():
                    pass

                qp_m1_carry = []  # prev stripe's deferred m1 qproj units
                for qs in range(NQS):
                    rf = recip_fillers(qs - 1, dve=True) if qs > 0 else []
                    qp = qproj_fillers(qs + 1) if qs < NQS - 1 else []
                    if qs == 0:
                        # remaining input projections ride along stripe 0
                        # (2 filler pops per kt), ordered so every tile's
                        # drain is emitted before its first consumer.
                        # m1 q-projections are deferred one stripe (QTs
                        # m1 isn't read until that stripe's hp1 block) to
                        # relieve stripe 0's structural PE overload.
                        fillers.extend(kq_fillers(1, 0))
                        for tb in range(6, 10):
                            fillers.extend(vtb_fillers(tb))
                        fillers.extend(kq_fillers(2, 0))
                        fillers.extend(kq_fillers(3, 0))
                        for tb in range(10, NKT):
                            fillers.extend(vtb_fillers(tb))
                        for qh in range(1, 4):
                            fillers.extend(kq_fillers(qh, 1))
                        fillers.extend(qp[0:8])
                        qp_m1_carry = qp[8:16]
                    elif rf:
                        # DVE reciprocal first (pops at step 0; the divide
                        # runs on the half-idle DVE while the carried m1
                        # qproj keeps the PE fed), bcasts well past the
                        # iterative-divide latency.
                        fillers.append(rf[0])
                        fillers.extend(qp_m1_carry)  # prev stripe m1
                        if qp:
                            fillers.extend(qp[0:8])  # qproj m0 (pins psF)
                        elif not qp_m1_carry:
                            fillers.extend([spacer] * 8)
                        fillers.extend(rf[1:])      # bcasts
                        fillers.extend(outproj_fillers(qs - 1))
                        qp_m1_carry = qp[8:16]
                    else:
                        fillers.extend(qp)

                    rs_t = rs_pool.tile([128, QW], F32, tag="rs",
                                        name=f"rs{qs}")
                    nc.gpsimd.memset(rs_t[:], 1.0)
                    rs_tiles[qs] = rs_t

                    for hp in range(2):
                        o_ps = [psO.tile([128, QW], F32, tag="psO",
                                         name=f"o{qs}_{hp}_{i}")
                                for i in range(2)]
                        for kt in range(NKT):
                            sc = psS.tile([128, 2 * QW], F32, tag="psS",
                                          name=f"s{qs}_{hp}_{kt}")
                            for hh in range(2):
                                lo = hh * DK
                                nc.tensor.matmul(
                                    sc[:, hh * QW:(hh + 1) * QW],
                                    KT[hp][lo:lo + DK,
                                           kt * 128:(kt + 1) * 128],
                                    QTs[qs][hp][lo:lo + DK, :],
                                    start=True, stop=True)
                            es = es_pool.tile([128, 2 * QW], BF16, tag="es",
                                              name=f"e{qs}_{hp}_{kt}")
                            # scores carry the WSCALE^2 from the fp8
                            # weight pre-scaling; fold it out here exactly
                            nc.scalar.activation(
                                es[:], sc[:], AFT.Exp,
                                scale=float(SCALE / (WSCALE * WSCALE)))
                            # flush older blocks now; hold up to 3 of the
                            # current block while kt < 3
                            while pending and pending[0][0:2] != (qs, hp):
                                flush_one()
                            pending.append((qs, hp, o_ps, es, kt))
                            target = 3 if kt < 3 else (2 if kt < 5 else 1)
                            while len(pending) > target:
                                flush_one()
                            # double-pop where the filler queue must
                            # drain fast (stripe-0 hp0 carries 32 units
                            # in 16 steps) and near stripe seams so
                            # leftovers don't flush serially; stripe-0
                            # hp1 has only 20 units since the m1-qproj
                            # carry, so single pops there keep the PE at
                            # ~1.07us/step instead of starving the exp
                            # stream at 1.49
                            npop = 2 if ((qs == 0 and hp == 0) or
                                         (hp == 1 and kt >= NKT - 4)) else 1
                            for _ in range(npop):
                                if fillers:
                                    fillers.popleft()()
                    # leftover fillers must land before the next stripe's
                    # scores read tiles they write (QTs of qs+1)
                    while fillers:
                        fillers.popleft()()

                # tail: flush last aV + drains, hp1 dance, outproj.
                # ~4.3us of dependency-free matmuls span the PE-idle
                # reciprocal-dance window so the HAM clock gate stays at
                # 2.4 GHz for the final output-projection matmuls.
                while pending:
                    flush_one()
                # sized to the ~3us reciprocal-dance window -- fewer and
                # the HAM MID window fires (cold outproj MMs, ~+2us);
                # more delays the bcast at the FIFO head
                warm2 = psF.tile([128, QW], F32, tag="psF", name="warm2")
                for i in range(14):
                    nc.tensor.matmul(
                        warm2[0:DK, :], ones_bf[:, 0:DK], KT[0][:, 0:QW],
                        start=True, stop=True)
                for f in recip_fillers(NQS - 1, hps=(1,)):
                    f()
                # (a second warm burst after the bcast was tried for the
                # ~1.5us of HAM-cold outproj MMs and cost +15us -- the
                # Tile scheduler reshuffles global emission around any
                # insertion here; leave the tail alone)
                for f in outproj_fillers(NQS - 1):
                    f()

    from concourse.bacc import get_activation_tables
    import bass_rust as _br
    _combined = "natural_log_exp_and_others"
    _tabs = []
    for _name, _fns in get_activation_tables(nc.m.arch).items():
        if _name != _combined:
            _fns = _fns - {AFT.Exp, AFT.Ln}
        _tabs.append((_name, _fns))
    _br.insert_act_table_loads(nc, _tabs)
    nc.compile()
    return nc


def _numpy_reference(q, k, v, mask, Wq, bq, Wk, bk, Wv, bv, Wo, bo):
    """Fallback for a non-trivial mask (never hit with the stock inputs)."""
    Bn, Tn, _ = q.shape
    H, dk = HEADS, DK

    def split(x):
        return x.reshape(Bn, Tn, H, dk).transpose(0, 2, 1, 3)

    qh = split(q @ Wq + bq)
    kh = split(k @ Wk + bk)
    vh = split(v @ Wv + bv)
    s = np.einsum("bhqd,bhkd->bhqk", qh, kh) / np.sqrt(np.float32(dk))
    s = np.where(mask, s, -np.inf)
    s = s - s.max(axis=-1, keepdims=True)
    e = np.exp(s)
    a = e / e.sum(axis=-1, keepdims=True)
    o = np.einsum("bhqk,bhkd->bhqd", a, vh)
    o = o.transpose(0, 2, 1, 3).reshape(Bn, Tn, H * dk)
    return (o @ Wo + bo).astype(np.float32)


def kernel(q, k, v, mask, Wq, bq, Wk, bk, Wv, bv, Wo, bo):
    global LAST_RESULTS
    q = np.asarray(q, np.float32)
    k = np.asarray(k, np.float32)
    v = np.asarray(v, np.float32)
    mask = np.asarray(mask, bool)
    Wq, bq = np.asarray(Wq, np.float32), np.asarray(bq, np.float32)
    Wk, bk = np.asarray(Wk, np.float32), np.asarray(bk, np.float32)
    Wv, bv = np.asarray(Wv, np.float32), np.asarray(bv, np.float32)
    Wo, bo = np.asarray(Wo, np.float32), np.asarray(bo, np.float32)

    if not mask.all():
        return _numpy_reference(q, k, v, mask, Wq, bq, Wk, bk, Wv, bv, Wo, bo)

    nc = _build_program()

    # host-side sharding; activations packed chunk-major per column
    # group (see the dram parameter comments in _build_program)
    def pack_cols(xT_b, w):
        ng = T // w
        return np.ascontiguousarray(
            xT_b.reshape(NF, 128, ng, w).transpose(2, 1, 0, 3)
            .reshape(ng, 128, NF * w))

    xP = {}
    for b in range(B):
        xq_t, xk_t, xv_t = (x[b].T.astype(BF) for x in (q, k, v))
        xP[b] = (pack_cols(xq_t, QW), pack_cols(xk_t, QW),
                 pack_cols(xv_t, 128))

    def w_chunks(W, g):
        # (1024, 256) head-group slice -> [128, 8*256] chunk-major layout,
        # pre-scaled by WSCALE so fp8 e4m3 quantization stays out of the
        # subnormal range (raw std 1/32; scaled std 1/4).  The scale is
        # folded out exactly: exp(scale=SCALE/WSCALE^2) on the QK path,
        # Wo/WSCALE on the V path.
        Wg = W[:, g * GD:(g + 1) * GD] * np.float32(WSCALE)
        return np.ascontiguousarray(
            Wg.reshape(NF, 128, GD).transpose(1, 0, 2)
            .reshape(128, NF * GD).astype(BF))

    in_maps = []
    for c in range(NCORES):
        b, g = divmod(c, GH)
        xq_t, xk_t, xv_t = xP[b]
        in_maps.append({
            "xq": xq_t, "xk": xk_t, "xv": xv_t,
            "wq": w_chunks(Wq, g), "wk": w_chunks(Wk, g),
            "wv": w_chunks(Wv, g),
            "wo": np.ascontiguousarray(
                (Wo[g * GD:(g + 1) * GD, :] / np.float32(WSCALE))
                .astype(BF)).reshape(2, 128, D),
            "bqv": np.ascontiguousarray(
                (bq[g * GD:(g + 1) * GD] * np.float32(WSCALE))
                .reshape(2, 128).T),
        })

    LAST_RESULTS = run_bass_kernel_spmd(
        nc, in_maps, list(range(NCORES)),
        trace=bool(os.environ.get("KERNEL_TRACE")))
    res = LAST_RESULTS.results

    const_row = (bv @ Wo + bo).astype(np.float32)  # attn rows sum to 1
    full = np.empty((B, T, D), np.float32)
    for b in range(B):
        acc = res[b * GH]["out"].astype(np.float32)
        for g in range(1, GH):
            acc = acc + res[b * GH + g]["out"].astype(np.float32)
        full[b] = acc + const_row
    return full



# revision 39
# speedup vs baseline: 1.0821x; 1.0821x over previous
"""Trainium2 Bass kernel: 16-head MHA (B=2, T=2048, D=1024, d_k=64).

Sharding (8 NeuronCores): data-parallel over the batch (2) x tensor-parallel
over head groups (4 groups of 4 heads).  Core c handles batch b = c//4 and
heads [4g, 4g+4) with g = c%4.  Each core computes its partial output
    sum_{h in group} softmax((q Wq_h + bq_h)(k Wk_h)^T / 8) (v Wv_h) Wo_h
and the host sums the 4 partials per batch and adds the constant row
bo + bv @ Wo once.  bk is dropped: with the all-ones mask it shifts every
score row by a per-row constant, which softmax ignores exactly.

Design notes (420us baseline -> ~211us):
  * every matmul operand is bf16 (FWL weight loads, fp32 PSUM
    accumulate); output DMA'd as bf16 and upconverted host-side.
    The two heads of a pair sit on partition halves 0:64 / 64:128, so
    their C=64 scores matmuls land on disjoint PE row-tiles (T0/T8) and
    execute CONCURRENTLY (measured: starts 3 ns apart) -- scores cost
    half the naive streaming time.  (Column-tiled pair splits of the
    C=128 projections were tried and serialize; only row tiles overlap.)
  * V is projected directly in [t, v-col] layout (stationary = x^T
    chunk, moving = Wv) -- no PE transposes.  Activations arrive via
    host-packed chunk-major layouts (xq/xk per 512-col group, xv per
    128-row k-tile) so each projection unit depends on ~1 MB of DMA,
    not the whole tensor.
  * attention runs in 512-wide q stripes; per (stripe, head-pair, kt):
    2 concurrent scores MMs -> one [128,1024] fp32 PSUM tile, one ACT
    exp -> bf16 es, 2 aV MMs accumulating into per-head [65,512] PSUM
    (the 65th V_ext ones-column yields softmax rowsums for free, which
    is column-optimal: a separate rowsum matmul would stream es again).
    PSUM: scores dbuf 2x2 banks + O-accum 2 + filler 2 = 8 exactly.
  * one flat software-pipelined (stripe, hp, kt) stream: aV runs 1-3
    steps behind scores/exp (deepened at block starts) so the FIFO PE
    queue never waits on an exp or on the previous block's drains.
  * the PE is kept dense -- and the HAM clock gate at 2.4 GHz -- by
    emitting everything else as fillers inside the ACT-bound attention
    loop: K groups 1-3 + V tiles 6-15 + Q stripe s+1 projections, the
    reciprocal dance, and stripe s-1's output projection, each unit
    ordered so its drain is emitted before its first consumer.
    ~150 dependency-free warm-up matmuls cover the ~7us DMA startup.
  * softmax denominators: rowsum rows are collected at partitions 32h;
    one batched ACT Ln + Exp(-1) computes all reciprocals directly on
    that layout (ACT cost depends only on per-lane depth, so no
    transposes are needed), then one C=128 selector matmul per head
    pair broadcasts them across partitions.  The last stripe runs
    hp0's reciprocal under hp1's attention and drains on the
    otherwise-idle ACT queue to shorten the serial tail.

Session 2 (209.9us -> ~204us) landed: DVE reciprocal offload for the
mid-stream stripes (nc.vector.reciprocal f32->bf16 under
allow_low_precision, popped at deque slot 0 with bcasts ~9 slots
later; ACT Ln+Exp kept for the latency-critical last-stripe dances),
m1 q-projections carried one stripe later (stripe 0 is ~6us
PE-overloaded structurally), halved wq/xq0/wk/xk0 transfers with
half-wave phase-A projection (first exp 31.7 -> 29.6us), rowsum rows
drained before u rows in the last block, warm2 sized to the dance.

Structure (trace-measured): first exp ~29.6us | 128 exps at ~1.12us
cadence, stripes 1-3 fully dense, stripe-0 ~13us of ACT gaps (~6us
structural PE overload + ~7us DMA-arrival JIT misses) | tail ~14us.
ACT busy ~148us IS the floor: (N+352)/1.2 ns per ACTIVATE, so 128
[128,1024] exps can't go below ~147us.

DEAD ENDS (all HW-measured, do not retry):
  * DMA multi-queue: sync+scalar HWDGE and gpsimd SWDGE queues all
    share ~220GB/s with PACKET round-robin -- queue choice cannot
    express priority, so any concurrent queue dilutes the critical
    phase-A set.  2-way dealt, 3-way dealt, critical-first-per-queue,
    and gated-SWDGE side channels measured +4..+29us vs one sync
    queue in strict need order.  Completion sems also fire ~5-7us
    after a transfer's last packet under load.
  * fp8 (e4m3) anywhere in the multiply chain: measured 7.0e-2 rel
    err (gate 2e-2).  Softmax/projection weight noise does NOT
    sqrt(N)-average: perturbing weights against random values lands
    the full ~4-5% element noise directly on the output.  This also
    kills fp8-DoubleRow aV/es plans.
  * [128,2048] exps (to amortize the +352): needs 4-bank scores
    tiles double-buffered = 8 banks; O-accum (2, ones-column rowsum
    trick needs M=65 so two heads cannot share a bank) + fillers (2)
    don't fit.  Single-buffered 4-bank scores serializes PE vs ACT
    (period 2.85us/2kt vs 2.24) -- worse.
  * merging the two per-hp OT normalize muls into one [128,512] op
    via a combined ub tile (hh1 drained to partitions 64:128 with a
    partition-SHIFTED tensor_copy): correct but +16us -- the shifted
    copy/mul lowers to something far slower than the aligned form.
    Keep drains partition-aligned; only the dst may be offset (the
    existing OT-mul / rowsum-copy patterns).
  * ANY local reshuffle of the tuned schedule regressed ~15us even
    when the local theory was sound: 6 extra warm MMs inserted after
    the last dance's bcast (to bridge the HAM MID window) +15us; and
    stripe-0 hp1 single-pops (20 units / 32 pops, smoothing PE to
    1.07us/step) +17us.  The Tile scheduler derives global emission
    order from dependencies+priorities, so inserting/deferring
    instructions perturbs far-away interleavings.  Treat the current
    filler/pop schedule as a fragile global optimum: change one
    thing at a time and re-measure end to end.

OPEN LEAD (big, unverified, ~15us): stage scores out of PSUM with
GPSIMD (idle all run) tensor_copy fp32->fp16 [128,1024] tiles into
an SBUF ring, then exp in [128,4096] batches (32x(4096+352)/1.2 =
119us of ACT vs 147).  fp16 staging keeps logit error ~0.4%; bf16 is
too coarse (3%).  Needs: gpsimd PSUM-read capability + per-op cost
verified, +~30KB/partition SBUF (es tiles grow to batch width),
aV flush lag raised to a full batch, and accepts stripes 1-3
becoming PE-bound at ~31us/stripe (net ~-15us).
"""

import functools
import os
from collections import deque

import ml_dtypes
import numpy as np

import concourse.bass as bass
import concourse.mybir as mybir
import concourse.tile as tile
from concourse import bacc
from concourse.bass_utils import run_bass_kernel_spmd

F32 = mybir.dt.float32
F32R = mybir.dt.float32r
BF16 = mybir.dt.bfloat16
F8 = mybir.dt.float8e4
AFT = mybir.ActivationFunctionType
BF = ml_dtypes.bfloat16
WSCALE = 1.0  # (fp8 input quantization was tried: ~7% output error --
#  softmax/projection weight noise does NOT sqrt-average away; any fp8
#  tensor in the multiply chain lands ~5-7% directly on the output)

D = 1024          # model dim
T = 2048          # sequence length
B = 2             # batch
HEADS = 16        # total heads
DK = 64           # head dim
NCORES = 8
GH = 4            # heads per core
GD = GH * DK      # 256 projection cols per core
NF = D // 128     # 8 contraction chunks
NKT = T // 128    # 16 k tiles
NQS = 4           # 512-wide q stripes
QW = T // NQS     # 512
SCALE = 1.0 / np.sqrt(np.float32(DK))  # 1/8

# Results of the last run (for test harness introspection: exec_time_ns etc.)
LAST_RESULTS = None


@functools.lru_cache(maxsize=1)
def _build_program():
    nc = bacc.Bacc("TRN2", target_bir_lowering=False, debug=False,
                   num_devices=NCORES)

    # host-packed activation layouts (see _pack_* in kernel()):
    #   xq[s]  = [128, NF*QW]  q-stripe s, chunk-major (8 KiB DMA lines)
    #   xk[qh] = [128, NF*QW]  k column-group qh, chunk-major
    #   xv[tb] = [128, NF*128] k-tile tb, chunk-major (2 KiB lines)
    xq = nc.declare_dram_parameter("xq", [NQS, 128, NF * QW], BF16,
                                   isOutput=False)
    xk = nc.declare_dram_parameter("xk", [4, 128, NF * QW], BF16,
                                   isOutput=False)
    xv = nc.declare_dram_parameter("xv", [NKT, 128, NF * 128], BF16,
                                   isOutput=False)
    wq = nc.declare_dram_parameter("wq", [128, NF * GD], BF16, isOutput=False)
    wk = nc.declare_dram_parameter("wk", [128, NF * GD], BF16, isOutput=False)
    wv = nc.declare_dram_parameter("wv", [128, NF * GD], BF16, isOutput=False)
    wo = nc.declare_dram_parameter("wo", [2, 128, D], BF16, isOutput=False)
    bqv = nc.declare_dram_parameter("bqv", [128, 2], F32, isOutput=False)
    out = nc.declare_dram_parameter("out", [T, D], BF16, isOutput=True)

    import contextlib
    with tile.TileContext(nc) as tc, contextlib.ExitStack() as _st:
        # ---- persistent pools -------------------------------------------
        def _pool(**kw):
            return _st.enter_context(tc.tile_pool(**kw))

        if True:
            kt_pool = _pool(name="kt", bufs=2)
            vext_pool = _pool(name="vext", bufs=NKT)
            qts_pool = _pool(name="qts", bufs=NQS * 2)
            ots_pool = _pool(name="ots", bufs=NQS * 2)
            w_pool = _pool(name="wts", bufs=3)
            wo_pool = _pool(name="wop", bufs=2)
            xq_pool = _pool(name="xq", bufs=NQS)
            xk_pool = _pool(name="xk", bufs=4)
            xv_pool = _pool(name="xv", bufs=NKT)
            const_pool = _pool(name="const", bufs=1)
            ones_f32 = const_pool.tile([128, GH], F32, tag="ones32")
            nc.gpsimd.memset(ones_f32[:], 1.0)
            ones_bf = const_pool.tile([128, DK], BF16, tag="onesbf")
            nc.gpsimd.memset(ones_bf[:], 1.0)
            bqv_sb = const_pool.tile([128, 2], F32, tag="bqv")

            KT = [kt_pool.tile([128, T], BF16, tag="kt", name=f"kt{m}")
                  for m in range(2)]
            VE = [vext_pool.tile([128, GH * (DK + 1)], BF16, tag="vext",
                                 name=f"ve{i}") for i in range(NKT)]
            # per-stripe Q^T and O^T tiles (heads of pair hp stacked 64+64)
            QTs = [[qts_pool.tile([128, QW], BF16, tag="qts",
                                  name=f"qt{s}_{m}") for m in range(2)]
                   for s in range(NQS)]
            OTs = [[ots_pool.tile([128, QW], BF16, tag="ots",
                                  name=f"ot{s}_{m}") for m in range(2)]
                   for s in range(NQS)]
            WO = [wo_pool.tile([128, D], BF16, tag="wop", name=f"wo{m}")
                  for m in range(2)]

            wq_sb = w_pool.tile([128, NF * GD], BF16, tag="w", name="wq_sb")
            wk_sb = w_pool.tile([128, NF * GD], BF16, tag="w", name="wk_sb")
            wv_sb = w_pool.tile([128, NF * GD], BF16, tag="w", name="wv_sb")
            XQs = [xq_pool.tile([128, NF * QW], BF16, tag="xq",
                                name=f"xqs{s}") for s in range(NQS)]
            XKq = [xk_pool.tile([128, NF * QW], BF16, tag="xk",
                                name=f"xkq{i}") for i in range(4)]
            XVt = [xv_pool.tile([128, NF * 128], BF16, tag="xv",
                                name=f"xvt{i}") for i in range(NKT)]

            # ---- DMA in, spread over all three DMA-capable queues so the
            # startup-critical 5 MB (wq,xq0,wk,xk0,wv,xv0-5 -- everything
            # phase A gates on) streams at aggregate HBM rate instead of
            # serializing one queue's per-transfer ~2us completion costs.
            #   scalar HWDGE: weights + even xv tiles (idle until the exp
            #     stream starts, which is long after these land)
            #   sync HWDGE:   activations needed by phase A + stripe 0
            #   gpsimd SWDGE: background tensors needed late (bqv by the
            #     Q0 drain, xq1 by stripe-0 hp1 fillers, wo by stripe 1)
            # SDMA engines round-robin the ACTIVE queues at packet
            # granularity, so queue choice cannot express priority --
            # whatever is enqueued anywhere flows concurrently.  Priority
            # therefore comes from ORDER, exactly like the baseline's
            # single queue; the second HWDGE queue (scalar -- idle until
            # the exp stream starts, and all issues happen up front) only
            # serves to overlap per-transfer fixed costs.  Transfers are
            # dealt to the two queues alternately in global need order so
            # arrival order tracks need order at ~2x effective rate.
            # (A 3-way split with bulk data on the gpsimd SWDGE queue was
            # tried and regressed 25us: everything round-robins, so the
            # phase-A set finished only when ALL front-loaded bytes did.)
            # All bulk transfers ride ONE sync-engine HWDGE queue in strict
            # need order.  Multi-queue variants (2-way and 3-way splits,
            # critical-first per queue, need-order dealing) were all tried
            # and regressed 11-29us: concurrent queues share the same
            # ~220 GB/s and round-robin at packet granularity, so extra
            # queues only dilute priority -- a single busy queue already
            # pipelines transfers through the 8 completion-sem lanes.
            # The first four transfers are halved so phase A's Q0/K0
            # projections pipeline under the second halves' transfers.
            HW = NF * QW // 2     # half of a chunk-major activation tile
            HG = NF * GD // 2     # half of a chunk-major weight tile
            nc.sync.dma_start(wq_sb[:, 0:HG], wq[:, 0:HG])
            nc.sync.dma_start(XQs[0][:, 0:HW], xq[0][:, 0:HW])
            nc.sync.dma_start(wk_sb[:, 0:HG], wk[:, 0:HG])
            nc.sync.dma_start(XKq[0][:, 0:HW], xk[0][:, 0:HW])
            nc.sync.dma_start(wq_sb[:, HG:], wq[:, HG:])
            nc.sync.dma_start(XQs[0][:, HW:], xq[0][:, HW:])
            nc.sync.dma_start(wk_sb[:, HG:], wk[:, HG:])
            nc.sync.dma_start(XKq[0][:, HW:], xk[0][:, HW:])
            nc.sync.dma_start(wv_sb[:], wv[:])
            for tb in range(6):
                nc.sync.dma_start(XVt[tb][:], xv[tb])
            nc.sync.dma_start(XKq[1][:], xk[1])
            for tb in range(6, 10):
                nc.sync.dma_start(XVt[tb][:], xv[tb])
            nc.sync.dma_start(XKq[2][:], xk[2])
            nc.sync.dma_start(XKq[3][:], xk[3])
            for tb in range(10, NKT):
                nc.sync.dma_start(XVt[tb][:], xv[tb])
            nc.sync.dma_start(XQs[1][:], xq[1])
            nc.sync.dma_start(WO[0][:], wo[0])
            nc.sync.dma_start(WO[1][:], wo[1])
            nc.sync.dma_start(XQs[2][:], xq[2])
            nc.sync.dma_start(XQs[3][:], xq[3])
            # tiny, and off the bulk queue's serial order.  (A gated
            # SWDGE side-channel for late tensors was tried twice: even
            # gated to start after phase A, its packets round-robin
            # against the sync queue's stripe-0 feed and cost ~4us.)
            nc.gpsimd.dma_start(bqv_sb[:], bqv[:])

            # head-pair selectors: sel[hp][c, m] = (c == 32*(hp*2 + m//64)),
            # i.e. r_bc[m, :] = rinvT[32*head(m), :] after the C=128 matmul
            sel = [const_pool.tile([128, 128], BF16, tag=f"sel{hp}",
                                   name=f"sel{hp}") for hp in range(2)]
            for hp in range(2):
                nc.gpsimd.memset(sel[hp][:], 0.0)
                for hh in range(2):
                    c = 32 * (hp * 2 + hh)
                    nc.vector.tensor_copy(
                        sel[hp][c:c + 1, hh * DK:(hh + 1) * DK],
                        ones_bf[0:1, 0:DK])

            # V_ext ones columns (persistent; written once, no DMA dep)
            for tb in range(NKT):
                ve_r = VE[tb][:].rearrange("p (h x) -> p h x", x=DK + 1)
                nc.vector.tensor_copy(
                    ve_r[:, :, DK:DK + 1],
                    ones_f32[:].rearrange("p (h x) -> p h x", x=1))

            # (A same-bank row-tiled pair variant of these projections --
            # start=False accumulation onto a pre-zeroed bank -- was
            # tried and hangs the device at runtime; keep full-C MMs.)
            def q_project(s, m, ps_q, fc):
                nc.tensor.matmul(
                    ps_q[:],
                    wq_sb[:, fc * GD + m * 128:fc * GD + (m + 1) * 128],
                    XQs[s][:, fc * QW:(fc + 1) * QW],
                    start=(fc == 0), stop=(fc == NF - 1))
                if fc == NF - 1:
                    nc.vector.tensor_scalar_add(
                        QTs[s][m][:], ps_q[:], bqv_sb[:, m:m + 1])

            def k_project(qh, m, ps_k, fc):
                nc.tensor.matmul(
                    ps_k[:],
                    wk_sb[:, fc * GD + m * 128:fc * GD + (m + 1) * 128],
                    XKq[qh][:, fc * QW:(fc + 1) * QW],
                    start=(fc == 0), stop=(fc == NF - 1))
                if fc == NF - 1:
                    nc.vector.tensor_copy(
                        KT[m][:, qh * QW:(qh + 1) * QW], ps_k[:])

            def v_project(tb, ps_v, dc):
                nc.tensor.matmul(
                    ps_v[:, 0:GD],
                    XVt[tb][:, dc * 128:(dc + 1) * 128],
                    wv_sb[:, dc * GD:(dc + 1) * GD],
                    start=(dc == 0), stop=(dc == NF - 1))
                if dc == NF - 1:
                    ve_r = VE[tb][:].rearrange("p (h x) -> p h x", x=DK + 1)
                    nc.vector.tensor_copy(
                        ve_r[:, :, 0:DK],
                        ps_v[:, 0:GD].rearrange("p (h x) -> p h x", x=DK))

            # ---- phase A: warm the HAM clock gate with dummy matmuls
            # (no DMA dependency), then project Q stripe 0, K group 0 and
            # V tiles 0-5 as their inputs land.
            with tc.tile_pool(name="psA", bufs=8,
                              space=bass.MemorySpace.PSUM) as psA:
                # ~6us of dependency-free matmuls: warms the HAM clock
                # gate AND covers the DMA/preamble startup so real
                # projections start the moment their data lands.  Q0/K0
                # run in half-tile waves matching the split DMAs above,
                # so projection overlaps the second half's transfer.
                warm = psA.tile([128, QW], F32, tag="psA", name="warm")
                for i in range(110):
                    nc.tensor.matmul(
                        warm[0:DK, 0:DK], ones_bf[:, 0:DK],
                        ones_bf[:, 0:DK], start=True, stop=True)
                ps_q = [psA.tile([128, QW], F32, tag="psA", name=f"psq{m}")
                        for m in range(2)]
                ps_k = [psA.tile([128, QW], F32, tag="psA", name=f"psk0_{m}")
                        for m in range(2)]
                for half in range(2):
                    for m in range(2):
                        for fc in range(4 * half, 4 * half + 4):
                            q_project(0, m, ps_q[m], fc)
                for half in range(2):
                    for m in range(2):
                        for fc in range(4 * half, 4 * half + 4):
                            k_project(0, m, ps_k[m], fc)
                for tb in range(6):
                    ps_v = psA.tile([128, QW], F32, tag="psA",
                                    name=f"psv{tb}")
                    for dc in range(NF):
                        v_project(tb, ps_v, dc)

            # ---- phase B: striped attention with PE fillers -------------
            with contextlib.ExitStack() as _stB:
                def _poolB(**kw):
                    return _stB.enter_context(tc.tile_pool(**kw))

                es_pool = _poolB(name="ep", bufs=5)
                ub_pool = _poolB(name="ubp", bufs=8)
                rs_pool = _poolB(name="rsp", bufs=2)
                ob_pool = _poolB(name="obp", bufs=4)
                psS = _poolB(name="psS", bufs=2,
                             space=bass.MemorySpace.PSUM)
                psO = _poolB(name="psO", bufs=2,
                             space=bass.MemorySpace.PSUM)
                psF = _poolB(name="psF", bufs=2,
                             space=bass.MemorySpace.PSUM)
                ub_tiles = {}     # (qs, hp, hh) -> [64, 512] f32 tile
                rs_tiles = {}     # qs -> [128, 512] f32 rowsum-spread tile

                fstate = {}

                def qproj_fillers(s):
                    fs = []
                    for m in range(2):
                        def mk(mm, fc):
                            def f():
                                if fc == 0:
                                    fstate['q', mm] = psF.tile(
                                        [128, QW], F32, tag="psF",
                                        name=f"psq{s}_{mm}")
                                q_project(s, mm, fstate['q', mm], fc)
                            return f
                        for fc in range(NF):
                            fs.append(mk(m, fc))
                    return fs

                def kq_fillers(qh, m):
                    """K projection of column-group qh, head-pair tile m
                    (2 chunk-pairs per filler)."""
                    def mk(fp):
                        def f():
                            if fp == 0:
                                fstate['k', qh, m] = psF.tile(
                                    [128, QW], F32, tag="psF",
                                    name=f"psk{qh}_{m}")
                            for fc in (2 * fp, 2 * fp + 1):
                                k_project(qh, m, fstate['k', qh, m], fc)
                        return f
                    return [mk(fp) for fp in range(4)]

                def vtb_fillers(tb):
                    """V projection of k-tile tb (4 chunk-pairs/filler)."""
                    def mk(dp):
                        def f():
                            if dp == 0:
                                fstate['v', tb] = psF.tile(
                                    [128, QW], F32, tag="psF",
                                    name=f"psv{tb}")
                            for dc in range(4 * dp, 4 * dp + 4):
                                v_project(tb, fstate['v', tb], dc)
                        return f
                    return [mk(0), mk(1)]

                def recip_fillers(s, hps=(0, 1), dve=False):
                    """Reciprocal + normalize for stripe s (rowsums at
                    partitions 32h of rs_tiles[s]).  ACT cost depends only
                    on per-lane depth (512), so Ln/Exp run directly on the
                    [32h, q] layout and the selector matmul consumes the
                    result as-is -- no transposes needed.  The caller must
                    space fs[0] a few pops after the rowsum drains.

                    dve=True computes the reciprocal on the DVE instead
                    (iterative divide, ~2-4us/tile) -- the DVE is ~50%
                    idle mid-kernel while the ACT queue is the bottleneck,
                    so stripes consumed mid-stream use this path and keep
                    the ACT Ln+Exp only for the latency-critical
                    last-stripe dances.  The caller must emit the bcast
                    fillers several pops after fs[0] to cover the divide
                    latency."""
                    fs = []

                    if dve:
                        def t2():
                            rinv = rs_pool.tile([128, QW], BF16, tag="rinv",
                                                name=f"rinv{s}")
                            with nc.allow_low_precision(
                                    reason="softmax denom bf16"):
                                nc.vector.reciprocal(rinv[:], rs_tiles[s][:])
                            recip_fillers.rinvT = rinv
                    else:
                        def t2():
                            lnr = rs_pool.tile([128, QW], F32, tag="rsT",
                                               name=f"lnr{s}")
                            nc.scalar.activation(lnr[:], rs_tiles[s][:],
                                                 AFT.Ln)
                            rinv = rs_pool.tile([128, QW], BF16, tag="rinv",
                                                name=f"rinv{s}")
                            nc.scalar.activation(rinv[:], lnr[:],
                                                 AFT.Exp, scale=-1.0)
                            recip_fillers.rinvT = rinv
                    fs.append(t2)

                    def mk_bcast(hp):
                        def f():
                            r_bc = psF.tile([128, QW], F32, tag="psF",
                                            name=f"rbc{s}_{hp}")
                            nc.tensor.matmul(
                                r_bc[:],
                                sel[hp][:],
                                recip_fillers.rinvT[:],
                                start=True, stop=True)
                            for hh in range(2):
                                nc.vector.tensor_mul(
                                    OTs[s][hp][hh * DK:(hh + 1) * DK, :],
                                    ub_tiles.pop((s, hp, hh))[0:DK, :],
                                    r_bc[hh * DK:(hh + 1) * DK, :])
                        return f
                    for hp in hps:
                        fs.append(mk_bcast(hp))
                    return fs
                recip_fillers.rinvT = None

                def outproj_fillers(s):
                    """Each (tt, ei) unit is split into a matmul closure
                    and a drain closure so the PE filler bursts stay
                    fine-grained inside the exp-bound attention cadence."""
                    fs = []

                    def mk_mm(tt, ei):
                        def f():
                            if ei == 0:
                                outproj_fillers.ob = ob_pool.tile(
                                    [128, D], BF16, tag="ob",
                                    name=f"ob{s}_{tt}")
                            f_ps = psF.tile([128, QW], F32, tag="psF",
                                            name=f"fps{s}_{tt}_{ei}")
                            fstate['op'] = f_ps
                            for m in range(2):
                                nc.tensor.matmul(
                                    f_ps[:],
                                    OTs[s][m][:, tt * 128:(tt + 1) * 128],
                                    WO[m][:, ei * QW:(ei + 1) * QW],
                                    start=(m == 0), stop=(m == 1))
                        return f

                    def mk_drain(tt, ei):
                        def f():
                            ob = outproj_fillers.ob
                            f_ps = fstate.pop('op')
                            if s == NQS - 1 and (tt + ei) % 2 == 0:
                                # tail: alternate drains between the idle
                                # ACT queue and DVE so they run 2-wide
                                nc.scalar.activation(
                                    ob[:, ei * QW:(ei + 1) * QW], f_ps[:],
                                    AFT.Copy)
                            else:
                                nc.vector.tensor_copy(
                                    ob[:, ei * QW:(ei + 1) * QW], f_ps[:])
                            t0 = (s * 4 + tt) * 128
                            if s == NQS - 1:
                                # tail: per-half DMAs fire right after
                                # their own drain, so the final transfer
                                # (which gates kernel end) starts earlier
                                nc.sync.dma_start(
                                    out[t0:t0 + 128, ei * QW:(ei + 1) * QW],
                                    ob[:, ei * QW:(ei + 1) * QW])
                            elif ei == 1:
                                nc.sync.dma_start(out[t0:t0 + 128, :], ob[:])
                        return f
                    for tt in range(4):
                        for ei in range(2):
                            fs.append(mk_mm(tt, ei))
                            fs.append(mk_drain(tt, ei))
                    return fs
                outproj_fillers.ob = None

                # flat (qs, hp, kt) stream: aV is emitted 1-3 steps behind
                # scores/exp so the FIFO PE queue never waits on an exp
                # before issuing independent scores work.  At block starts
                # the hold-back deepens to 3 so the previous block's DVE
                # drains (which gate aV(kt0) via o_ps buffer reuse) finish
                # under the run-ahead scores instead of stalling the PE.
                fillers = deque()
                pending = deque()  # (qs, hp, o_ps, es, kt)

                def flush_one():
                    pqs, php, po_ps, pes, pkt = pending.popleft()
                    for hh in range(2):
                        h = php * 2 + hh
                        nc.tensor.matmul(
                            po_ps[hh][0:DK + 1, :],
                            VE[pkt][:, h * (DK + 1):(h + 1) * (DK + 1)],
                            pes[:, hh * QW:(hh + 1) * QW],
                            start=(pkt == 0), stop=(pkt == NKT - 1))
                    if pkt == NKT - 1:
                        # drain O^T + rowsum row; heads at partitions 32h.
                        # The very last block's drains go on the otherwise
                        # idle ACT queue to shorten the serial tail.
                        last = pqs == NQS - 1 and php == 1
                        if last:
                            # rowsum rows first (they gate the tail Ln),
                            # u drains after, 2-wide on DVE + idle ACT.
                            us = [ub_pool.tile([128, QW], F32, tag="ub",
                                               name=f"ub{pqs}_{php}_{hh}")
                                  for hh in range(2)]
                            h0, h1 = php * 2, php * 2 + 1
                            nc.vector.tensor_copy(
                                rs_tiles[pqs][32 * h0:32 * h0 + 1, :],
                                po_ps[0][DK:DK + 1, :])
                            nc.scalar.activation(
                                rs_tiles[pqs][32 * h1:32 * h1 + 1, :],
                                po_ps[1][DK:DK + 1, :], AFT.Copy)
                            nc.vector.tensor_copy(
                                us[0][0:DK, :], po_ps[0][0:DK, :])
                            nc.scalar.activation(
                                us[1][0:DK, :], po_ps[1][0:DK, :],
                                AFT.Copy)
                            for hh in range(2):
                                ub_tiles[(pqs, php, hh)] = us[hh]
                        else:
                            for hh in range(2):
                                h = php * 2 + hh
                                u = ub_pool.tile([128, QW], F32, tag="ub",
                                                 name=f"ub{pqs}_{php}_{hh}")
                                nc.vector.tensor_copy(
                                    u[0:DK, :], po_ps[hh][0:DK, :])
                                nc.vector.tensor_copy(
                                    rs_tiles[pqs][32 * h:32 * h + 1, :],
                                    po_ps[hh][DK:DK + 1, :])
                                ub_tiles[(pqs, php, hh)] = u
                        if pqs == NQS - 1 and php == 0:
                            # last stripe: overlap hp0's half of the
                            # reciprocal under hp1's attention
                            fillers.extend([spacer] * 3)
                            fillers.extend(recip_fillers(pqs, hps=(0,)))

                def spacer():
                    pass

                qp_m1_carry = []  # prev stripe's deferred m1 qproj units
                for qs in range(NQS):
                    rf = recip_fillers(qs - 1, dve=True) if qs > 0 else []
                    qp = qproj_fillers(qs + 1) if qs < NQS - 1 else []
                    if qs == 0:
                        # remaining input projections ride along stripe 0
                        # (2 filler pops per kt), ordered so every tile's
                        # drain is emitted before its first consumer.
                        # m1 q-projections are deferred one stripe (QTs
                        # m1 isn't read until that stripe's hp1 block) to
                        # relieve stripe 0's structural PE overload.
                        fillers.extend(kq_fillers(1, 0))
                        for tb in range(6, 10):
                            fillers.extend(vtb_fillers(tb))
                        fillers.extend(kq_fillers(2, 0))
                        fillers.extend(kq_fillers(3, 0))
                        for tb in range(10, NKT):
                            fillers.extend(vtb_fillers(tb))
                        for qh in range(1, 4):
                            fillers.extend(kq_fillers(qh, 1))
                        fillers.extend(qp[0:8])
                        qp_m1_carry = qp[8:16]
                    elif rf:
                        # DVE reciprocal first (pops at step 0; the divide
                        # runs on the half-idle DVE while the carried m1
                        # qproj keeps the PE fed), bcasts well past the
                        # iterative-divide latency.
                        fillers.append(rf[0])
                        fillers.extend(qp_m1_carry)  # prev stripe m1
                        if qp:
                            fillers.extend(qp[0:8])  # qproj m0 (pins psF)
                        elif not qp_m1_carry:
                            fillers.extend([spacer] * 8)
                        fillers.extend(rf[1:])      # bcasts
                        fillers.extend(outproj_fillers(qs - 1))
                        qp_m1_carry = qp[8:16]
                    else:
                        fillers.extend(qp)

                    rs_t = rs_pool.tile([128, QW], F32, tag="rs",
                                        name=f"rs{qs}")
                    nc.gpsimd.memset(rs_t[:], 1.0)
                    rs_tiles[qs] = rs_t

                    for hp in range(2):
                        o_ps = [psO.tile([128, QW], F32, tag="psO",
                                         name=f"o{qs}_{hp}_{i}")
                                for i in range(2)]
                        for kt in range(NKT):
                            sc = psS.tile([128, 2 * QW], F32, tag="psS",
                                          name=f"s{qs}_{hp}_{kt}")
                            for hh in range(2):
                                lo = hh * DK
                                nc.tensor.matmul(
                                    sc[:, hh * QW:(hh + 1) * QW],
                                    KT[hp][lo:lo + DK,
                                           kt * 128:(kt + 1) * 128],
                                    QTs[qs][hp][lo:lo + DK, :],
                                    start=True, stop=True)
                            es = es_pool.tile([128, 2 * QW], BF16, tag="es",
                                              name=f"e{qs}_{hp}_{kt}")
                            # scores carry the WSCALE^2 from the fp8
                            # weight pre-scaling; fold it out here exactly
                            nc.scalar.activation(
                                es[:], sc[:], AFT.Exp,
                                scale=float(SCALE / (WSCALE * WSCALE)))
                            # flush older blocks now; hold up to 3 of the
                            # current block while kt < 3
                            while pending and pending[0][0:2] != (qs, hp):
                                flush_one()
                            pending.append((qs, hp, o_ps, es, kt))
                            target = 3 if kt < 3 else (2 if kt < 5 else 1)
                            while len(pending) > target:
                                flush_one()
                            # double-pop near the stripe seam so leftover
                            # fillers don't flush serially between the
                            # last aV and the next stripe's first scores
                            npop = 2 if (qs == 0 or
                                         (hp == 1 and kt >= NKT - 4)) else 1
                            for _ in range(npop):
                                if fillers:
                                    fillers.popleft()()
                    # leftover fillers must land before the next stripe's
                    # scores read tiles they write (QTs of qs+1)
                    while fillers:
                        fillers.popleft()()

                # tail: flush last aV + drains, hp1 dance, outproj.
                # ~4.3us of dependency-free matmuls span the PE-idle
                # reciprocal-dance window so the HAM clock gate stays at
                # 2.4 GHz for the final output-projection matmuls.
                while pending:
                    flush_one()
                # sized to the ~3us reciprocal-dance window -- fewer and
                # the HAM MID window fires (cold outproj MMs, ~+2us);
                # more delays the bcast at the FIFO head
                warm2 = psF.tile([128, QW], F32, tag="psF", name="warm2")
                for i in range(14):
                    nc.tensor.matmul(
                        warm2[0:DK, :], ones_bf[:, 0:DK], KT[0][:, 0:QW],
                        start=True, stop=True)
                for f in recip_fillers(NQS - 1, hps=(1,)):
                    f()
                for f in outproj_fillers(NQS - 1):
                    f()

    from concourse.bacc import get_activation_tables
    import bass_rust as _br
    _combined = "natural_log_exp_and_others"
    _tabs = []
    for _name, _fns in get_activation_tables(nc.m.arch).items():
        if _name != _combined:
            _fns = _fns - {AFT.Exp, AFT.Ln}
        _tabs.append((_name, _fns))
    _br.insert_act_table_loads(nc, _tabs)
    nc.compile()
    return nc


def _numpy_reference(q, k, v, mask, Wq, bq, Wk, bk, Wv, bv, Wo, bo):
    """Fallback for a non-trivial mask (never hit with the stock inputs)."""
    Bn, Tn, _ = q.shape
    H, dk = HEADS, DK

    def split(x):
        return x.reshape(Bn, Tn, H, dk).transpose(0, 2, 1, 3)

    qh = split(q @ Wq + bq)
    kh = split(k @ Wk + bk)
    vh = split(v @ Wv + bv)
    s = np.einsum("bhqd,bhkd->bhqk", qh, kh) / np.sqrt(np.float32(dk))
    s = np.where(mask, s, -np.inf)
    s = s - s.max(axis=-1, keepdims=True)
    e = np.exp(s)
    a = e / e.sum(axis=-1, keepdims=True)
    o = np.einsum("bhqk,bhkd->bhqd", a, vh)
    o = o.transpose(0, 2, 1, 3).reshape(Bn, Tn, H * dk)
    return (o @ Wo + bo).astype(np.float32)


def kernel(q, k, v, mask, Wq, bq, Wk, bk, Wv, bv, Wo, bo):
    global LAST_RESULTS
    q = np.asarray(q, np.float32)
    k = np.asarray(k, np.float32)
    v = np.asarray(v, np.float32)
    mask = np.asarray(mask, bool)
    Wq, bq = np.asarray(Wq, np.float32), np.asarray(bq, np.float32)
    Wk, bk = np.asarray(Wk, np.float32), np.asarray(bk, np.float32)
    Wv, bv = np.asarray(Wv, np.float32), np.asarray(bv, np.float32)
    Wo, bo = np.asarray(Wo, np.float32), np.asarray(bo, np.float32)

    if not mask.all():
        return _numpy_reference(q, k, v, mask, Wq, bq, Wk, bk, Wv, bv, Wo, bo)

    nc = _build_program()

    # host-side sharding; activations packed chunk-major per column
    # group (see the dram parameter comments in _build_program)
    def pack_cols(xT_b, w):
        ng = T // w
        return np.ascontiguousarray(
            xT_b.reshape(NF, 128, ng, w).transpose(2, 1, 0, 3)
            .reshape(ng, 128, NF * w))

    xP = {}
    for b in range(B):
        xq_t, xk_t, xv_t = (x[b].T.astype(BF) for x in (q, k, v))
        xP[b] = (pack_cols(xq_t, QW), pack_cols(xk_t, QW),
                 pack_cols(xv_t, 128))

    def w_chunks(W, g):
        # (1024, 256) head-group slice -> [128, 8*256] chunk-major layout,
        # pre-scaled by WSCALE so fp8 e4m3 quantization stays out of the
        # subnormal range (raw std 1/32; scaled std 1/4).  The scale is
        # folded out exactly: exp(scale=SCALE/WSCALE^2) on the QK path,
        # Wo/WSCALE on the V path.
        Wg = W[:, g * GD:(g + 1) * GD] * np.float32(WSCALE)
        return np.ascontiguousarray(
            Wg.reshape(NF, 128, GD).transpose(1, 0, 2)
            .reshape(128, NF * GD).astype(BF))

    in_maps = []
    for c in range(NCORES):
        b, g = divmod(c, GH)
        xq_t, xk_t, xv_t = xP[b]
        in_maps.append({
            "xq": xq_t, "xk": xk_t, "xv": xv_t,
            "wq": w_chunks(Wq, g), "wk": w_chunks(Wk, g),
            "wv": w_chunks(Wv, g),
            "wo": np.ascontiguousarray(
                (Wo[g * GD:(g + 1) * GD, :] / np.float32(WSCALE))
                .astype(BF)).reshape(2, 128, D),
            "bqv": np.ascontiguousarray(
                (bq[g * GD:(g + 1) * GD] * np.float32(WSCALE))
                .reshape(2, 128).T),
        })

    LAST_RESULTS = run_bass_kernel_spmd(
        nc, in_maps, list(range(NCORES)),
        trace=bool(os.environ.get("KERNEL_TRACE")))
    res = LAST_RESULTS.results

    const_row = (bv @ Wo + bo).astype(np.float32)  # attn rows sum to 1
    full = np.empty((B, T, D), np.float32)
    for b in range(B):
        acc = res[b * GH]["out"].astype(np.float32)
        for g in range(1, GH):
            acc = acc + res[b * GH + g]["out"].astype(np.float32)
        full[b] = acc + const_row
    return full

